# revision 24
# baseline (speedup 1.0000x reference)
"""Low-rank attention Trainium2 kernel (8 NeuronCores, SPMD), fp8 edition.

Math (reference):
    tmp = relu(x @ W.T + b); U,V,Z,T = split(tmp, 4, axis=1)
    norm = sum(U @ colsum(V)) / n + eps ;  D = 1/norm
    out = concat[(U @ (V.T @ Z)) * D, T]

Sharding: rows of x across 8 cores. Per-core partials (V.T@Z [k,k],
colsum(V), colsum(U)) are AllReduced on-device; each core then computes
its local U @ (VtZ) * D.

fp8 design (vs the 339us bf16 baseline):
- U, V, Z projections and V^T@Z run as fp8e4 DoubleRow matmuls (2 k-tiles
  per instruction, measured 216ns steady for moving-512 = true 2x bf16;
  LDWEIGHTS hides behind the previous matmul's streaming).
- x is quantized to fp8 on the HOST (x8 = e4m3(16x), 8MB/core, resident);
  on-device bf16->fp8 converts are not viable (only DVE writes fp8 fast).
- The T block stays bf16 (its error hits the output directly; fp8's ~2.5%
  elementwise would eat the whole 2e-2 budget). bf16 x streams through a
  rolling pool, one [1024, 512] block per T-pass block.
- Scales: x8 = 16x, W8 = 64W, vz fp8 = 32*[V|Z]; U drains unscaled (bf16,
  ACT relu scale 1/1024 + csu accum); VtZ psum = 1024 V^T Z | 32 csV.

Collective hiding (the bf16 baseline exposed ~41us of AllReduce):
- Phase 1a: V|Z + V^T@Z for ALL i-blocks first; V^T@Z accumulates across
  blocks in two PSUM chains (no per-block DVE adds). AllReduce A (the
  whole [k,k+1] x 2 payload) launches at ~1/3 of the kernel.
- Phase 1b: all U-passes; then AllReduce C (csu only, 1KB).
- T-passes follow (last TDEF read a csu-gated copy of the T-weights,
  pinning them after C's launch); phase-4 matmuls need only A's result
  (vtzr = V^T Z unscaled); the data-dependent D = 1/norm is applied at
  the phase-4 PSUM drains as a per-partition AP scale, so C's latency
  hides under the T-pass + phase-4 matmuls.
- DMA rings: x8 + T-out on sync, weights + staging + res-out on scalar,
  xb blocks + collectives on gpsimd.
"""
import sys

sys.path.insert(0, "/opt/trn_rl_repo")
import numpy as np
import ml_dtypes

BF16 = ml_dtypes.bfloat16
E4 = ml_dtypes.float8_e4m3

NCORES = 8
N_ROWS, D_IN, K = 65536, 1024, 256
NLOC = N_ROWS // NCORES      # 8192 rows per core
P = 128
IB = 512                     # i-block width
NB = NLOC // IB              # 16 blocks
EPS = 1e-6
TDEF = 6                     # T-pass blocks deferred behind AllReduce C
S_X, S_W, S_V = 16.0, 64.0, 32.0
S_R8 = 1.0 / 256  # VtZ fp8 scale (phase-4 DoubleRow)
X8CHUNKS = [(0, 512), (512, 512), (1024, 1024), (2048, 2048), (4096, 4096)]

_built = {}


def _build(d_rows):
    import concourse.bacc as bacc
    import concourse.mybir as mybir
    import concourse.tile as tile

    dt = mybir.dt
    f32, bf16, f8 = dt.float32, dt.bfloat16, dt.float8e4
    RELU = mybir.ActivationFunctionType.Relu
    DR = mybir.MatmulPerfMode.DoubleRow
    DT = d_rows // P
    KD2 = DT // 2            # DoubleRow kd-pairs
    KODD = DT % 2            # leftover plain-fp8 k-tile (bias-pad path)
    NSUB = IB // P
    SCL = 1.0 / (S_X * S_W)  # psum -> true pre-activation

    nc = bacc.Bacc("TRN2", target_bir_lowering=False, debug=False, num_devices=NCORES)
    NBX = NLOC // 1024       # x8 resident-load chunks
    x8d = nc.dram_tensor("x8", [NBX, P, DT, 1024], f8, kind="ExternalInput")
    xbd = nc.dram_tensor("xb", [NB, P, DT, IB], bf16, kind="ExternalInput")
    w8ud = nc.dram_tensor("w8u", [P, DT, K], f8, kind="ExternalInput")
    w8vzd = nc.dram_tensor("w8vz", [P, DT, 2 * K], f8, kind="ExternalInput")
    wttd = nc.dram_tensor("wtt", [P, DT, K], bf16, kind="ExternalInput")
    out = nc.dram_tensor("out", [NLOC, 2 * K], f32, kind="ExternalOutput")

    with tile.TileContext(nc) as tc:
        with (
            tc.tile_pool(name="wp", bufs=1) as wp,
            tc.tile_pool(name="xp", bufs=1) as xp,
            tc.tile_pool(name="xbp", bufs=8) as xbp,
            tc.tile_pool(name="up", bufs=1) as up,
            tc.tile_pool(name="vzp", bufs=6) as vzp,
            tc.tile_pool(name="ob", bufs=6) as ob,
            tc.tile_pool(name="acc", bufs=1) as accp,
            tc.tile_pool(name="ps", bufs=6, space="PSUM") as ps,
            tc.tile_pool(name="ps2", bufs=1, space="PSUM") as ps2,
            tc.tile_pool(name="dram", bufs=1, space="DRAM") as dram,
        ):
            # Weights (gpsimd/scalar rings), then resident x8 in per-kd
            # column chunks (sync ring, small first chunks so ib0 starts
            # early). bf16 x streams per-block via xbp below.
            w8vz = wp.tile([P, DT, 2 * K], f8, tag="w8vz")
            nc.scalar.dma_start(out=w8vz[:], in_=w8vzd[:])
            w8u = wp.tile([P, DT, K], f8, tag="w8u")
            nc.scalar.dma_start(out=w8u[:], in_=w8ud[:])
            xbts = {}

            def load_xb(ib):
                xbt = xbp.tile([P, DT, IB], bf16, tag="xb", name=f"xb{ib}")
                nc.scalar.dma_start(out=xbt[:], in_=xbd[ib])
                return xbt

            XC = 1024
            x8 = xp.tile([P, NBX, DT, XC], f8, tag="x8")
            nc.sync.dma_start(out=x8[:, 0, :, 0:IB], in_=x8d[0][:, :, 0:IB])
            nc.sync.dma_start(out=x8[:, 0, :, IB:XC], in_=x8d[0][:, :, IB:XC])
            for ci in range(1, NBX):
                q = nc.sync if ci % 2 == 0 else nc.scalar
                q.dma_start(out=x8[:, ci, :, :], in_=x8d[ci])
                if ci == 1:
                    wt = wp.tile([P, DT, K], bf16, tag="wt")
                    nc.scalar.dma_start(out=wt[:], in_=wttd[:])
                if 1 <= ci <= 7:
                    xbts[ci - 1] = load_xb(ci - 1)
            ones_row = wp.tile([1, P], f32, tag="ones_row")
            nc.vector.memset(ones_row[:], 1.0 / (S_V * S_R8))
            sync0 = accp.tile([1, 1], f32, tag="sync0")
            nc.vector.memset(sync0[:], 1.0)
            b0in = dram.tile([1, 1], f32)
            b0out = dram.tile([1, 1], f32)
            nc.scalar.dma_start(out=b0in[:], in_=sync0[:])
            nc.gpsimd.collective_compute(
                "AllReduce", mybir.AluOpType.add,
                replica_groups=[list(range(NCORES))],
                ins=[b0in.opt()], outs=[b0out.opt()],
            )

            ut8 = up.tile([P, 2, NLOC], f8, tag="ut8")
            csu_cols = [accp.tile([P, NB], f32, tag=f"csuc{h}", name=f"csuc{h}") for h in range(2)]

            def t_pass(ib, xbt, wsrc):
                """T = relu(x @ Wt): 4 row-subtiles, one batched out-DMA."""
                otb = ob.tile([P, NSUB, K], f32, tag="ob")
                for s in range(NSUB):
                    pt = ps.tile([P, K], f32, tag="work")
                    for kd in range(DT):
                        nc.tensor.matmul(
                            pt[:], xbt[:, kd, s * P:(s + 1) * P],
                            wsrc[:, kd, :],
                            start=(kd == 0), stop=(kd == DT - 1),
                        )
                    if s % 2 == 0:
                        nc.vector.tensor_relu(otb[:, s, :], pt[:])
                    else:
                        nc.scalar.activation(otb[:, s, :], pt[:], RELU)
                i0 = ib * IB
                nc.sync.dma_start(
                    out=out[i0:i0 + IB, K:2 * K].rearrange(
                        "(s p) c -> p s c", p=P),
                    in_=otb[:],
                )
                return otb

            # ---- phase 1a: V|Z fp8 projection + V^T@Z PSUM chains ----
            # vz col 512 = 1.0 rides the V^T@Z matmul to produce 32*csV in
            # column 256 of the [k, k+1] chain.
            pzh = [ps2.tile([P, K + 1], f32, tag=f"pz{h}", name=f"pz{h}") for h in range(2)]

            def vtz(ib, vz_tiles):
                for h in range(2):
                    for sp in range(NSUB // 2):
                        nc.tensor.matmul(
                            pzh[h][:], vz_tiles[sp][:, :, h * P:(h + 1) * P],
                            vz_tiles[sp][:, :, K:2 * K + 1],
                            start=(ib == 0 and sp == 0),
                            stop=(ib == NB - 1 and sp == NSUB // 2 - 1),
                            perf_mode=DR,
                        )

            prev_vz = None
            for ib in range(NB):
                vz_tiles = []
                for sp in range(NSUB // 2):
                    vzt = vzp.tile([P, 2, 2 * K + 16], f8, tag="vz")
                    for s2 in range(2):
                        s = sp * 2 + s2
                        pvz = ps.tile([P, IB], f32, tag="work")
                        ci, off = ib // 2, (ib % 2) * IB + s * P
                        for k2 in range(KD2):
                            nc.tensor.matmul(
                                pvz[:],
                                x8[:, ci, 2 * k2:2 * k2 + 2, off:off + P],
                                w8vz[:, 2 * k2:2 * k2 + 2, :],
                                start=(k2 == 0), stop=(k2 == KD2 - 1 and not KODD),
                                perf_mode=DR,
                            )
                        if KODD:
                            nc.tensor.matmul(
                                pvz[:],
                                x8[:, ci, DT - 1, off:off + P],
                                w8vz[:, DT - 1, :],
                                start=False, stop=True,
                            )
                        nc.vector.tensor_scalar(
                            out=vzt[:, s2, 0:2 * K], in0=pvz[:],
                            scalar1=S_V * SCL, scalar2=0.0,
                            op0=mybir.AluOpType.mult, op1=mybir.AluOpType.max,
                        )
                    nc.vector.memset(vzt[:, :, 2 * K:2 * K + 1], 1.0)
                    vz_tiles.append(vzt)
                if prev_vz is not None:
                    vtz(ib - 1, prev_vz)
                prev_vz = vz_tiles
            vtz(NB - 1, prev_vz)

            # ---- AllReduce A: the full V^T@Z | csV payload ----
            bin_a = dram.tile([2 * P, K + 1], f32)
            bout_a = dram.tile([2 * P, K + 1], f32)
            vtzs = [accp.tile([P, K + 1], f32, tag=f"vtzs{h}", name=f"vtzs{h}") for h in range(2)]
            for h in range(2):
                nc.vector.tensor_copy(vtzs[h][:], pzh[h][:])
                nc.scalar.dma_start(out=bin_a[h * P:(h + 1) * P, :], in_=vtzs[h][:])
            nc.gpsimd.collective_compute(
                "AllReduce", mybir.AluOpType.add,
                replica_groups=[list(range(NCORES))],
                ins=[bin_a.opt()], outs=[bout_a.opt()],
            )

            # ---- phase 1b: all U-passes (fp8 DR), then AllReduce C (csu);
            # the first 4 T-passes weave in so the xb stream + T-out DMA load
            # spreads over a ~100us window instead of one 74us burst ----
            for ib in range(NB):
                if ib % 4 == 3:
                    otb_last = t_pass(ib // 4, xbts.pop(ib // 4), wt)
                for h in range(2):
                    pu = ps.tile([P, IB], f32, tag="work")
                    ci, off = ib // 2, (ib % 2) * IB
                    for k2 in range(KD2):
                        nc.tensor.matmul(
                            pu[:], w8u[:, 2 * k2:2 * k2 + 2, h * P:(h + 1) * P],
                            x8[:, ci, 2 * k2:2 * k2 + 2, off:off + IB],
                            start=(k2 == 0), stop=(k2 == KD2 - 1 and not KODD),
                            perf_mode=DR,
                        )
                    if KODD:
                        nc.tensor.matmul(
                            pu[:], w8u[:, DT - 1, h * P:(h + 1) * P],
                            x8[:, ci, DT - 1, off:off + IB],
                            start=False, stop=True,
                        )
                    nc.vector.tensor_scalar(
                        out=ut8[:, h, ib * IB:(ib + 1) * IB], in0=pu[:],
                        scalar1=S_V * SCL, scalar2=0.0,
                        op0=mybir.AluOpType.mult, op1=mybir.AluOpType.max,
                    )
                    scr = ob.tile([P, IB], bf16, tag="uscr")
                    nc.scalar.activation(
                        scr[:], pu[:], RELU, scale=S_V * SCL,
                        accum_out=csu_cols[h][:, ib:ib + 1],
                    )

            csu = [accp.tile([P, 1], f32, tag=f"csu{h}", name=f"csu{h}") for h in range(2)]
            for h in range(2):
                nc.vector.reduce_sum(csu[h][:], csu_cols[h][:], axis=mybir.AxisListType.X)
            bin_c = dram.tile([2, P], f32)
            bout_c = dram.tile([2, P], f32)
            for h in range(2):
                nc.scalar.dma_start(
                    out=bin_c[h, 0:P].rearrange("(p one) -> p one", one=1),
                    in_=csu[h][:],
                )
            nc.gpsimd.collective_compute(
                "AllReduce", mybir.AluOpType.add,
                replica_groups=[list(range(NCORES))],
                ins=[bin_c.opt()], outs=[bout_c.opt()],
            )

            # ---- T-passes (xb streamed per block; last TDEF read wt2) ----
            for ib in range(4, NB):
                xbt = xbts.pop(ib) if ib in xbts else load_xb(ib)
                otb_last = t_pass(ib, xbt, wt)

            # ---- phase 3 (pinned AFTER the last T relu): the Tile scheduler
            # otherwise hoists these into the T window and head-of-line
            # blocks the DVE/ACT queues on AllReduce A for ~40us. gate1/gateS
            # are exact constants (1.0, SCL) with a true data dependency on
            # the final T-block's staging tile.
            gate1 = accp.tile([P, 1], f32, tag="gate1")
            nc.vector.tensor_scalar(
                out=gate1[:], in0=otb_last[:, 0, 0:1], scalar1=0.0, scalar2=1.0,
                op0=mybir.AluOpType.mult, op1=mybir.AluOpType.add,
            )
            gateS = accp.tile([P, 1], f32, tag="gateS")
            nc.vector.tensor_scalar_mul(gateS[:], gate1[:], SCL * S_R8)
            vtzf = [accp.tile([P, K + 1], f32, tag=f"vtzf{h}", name=f"vtzf{h}") for h in range(2)]
            for h in range(2):
                nc.gpsimd.dma_start(out=vtzf[h][:], in_=bout_a[h * P:(h + 1) * P, :])
            vtzr8 = accp.tile([P, 2, K], f8, tag="vtzr8")
            for h in range(2):
                nc.vector.tensor_scalar_mul(vtzr8[:, h, :], vtzf[h][:, 0:K], gateS[:])

            # D = 1/(csU.csV/n + eps) applied at the phase-4 drains
            csut = accp.tile([P, 2], f32, tag="csut")
            nc.gpsimd.dma_start(out=csut[:], in_=bout_c.rearrange("t p -> p t"))
            csvt = accp.tile([P, 2], f32, tag="csvt")
            for h in range(2):
                nc.vector.tensor_scalar_mul(csvt[:, h:h + 1], vtzf[h][:, K:K + 1], gate1[:])
            pdot = ps.tile([1, 1], f32, tag="work")
            for h in range(2):
                nc.tensor.matmul(
                    pdot[:], csut[:, h:h + 1], csvt[:, h:h + 1],
                    start=(h == 0), stop=(h == 1),
                )
            dsb = accp.tile([1, 1], f32, tag="dsb")
            nc.vector.tensor_scalar(
                out=dsb[:], in0=pdot[:], scalar1=1.0 / (S_V * S_V * N_ROWS), scalar2=EPS,
                op0=mybir.AluOpType.mult, op1=mybir.AluOpType.add,
            )
            nc.vector.reciprocal(dsb[:], dsb[:])
            pb = ps.tile([P, 1], f32, tag="work")
            nc.tensor.matmul(pb[:], ones_row[:], dsb[:], start=True, stop=True)
            dbc = accp.tile([P, 1], f32, tag="dbc")
            nc.vector.tensor_copy(dbc[:], pb[:])

            # ---- phase 4: res = (U @ VtZ) * D, batched row-natural writes ----
            # h-major over groups of 4 PSUM tiles: the moving operand stays
            # fixed for the group and each start/stop pair is spread apart,
            # keeping the weight path warm. D lands at the drains (AP scale).
            GG = 4
            for gb in range(NLOC // P // GG):
                prs = [ps.tile([P, K], f32, tag="work", name=f"pr{t}") for t in range(GG)]
                for t in range(GG):
                    i0 = (gb * GG + t) * P
                    nc.tensor.matmul(
                        prs[t][:], ut8[:, :, i0:i0 + P], vtzr8[:],
                        start=True, stop=True, perf_mode=DR,
                    )
                orb = ob.tile([P, GG, K], f32, tag="ob")
                for t in range(GG):
                    # split PSUM->SBUF scaled copies across DVE and ACT
                    if t % 2 == 0:
                        nc.vector.tensor_scalar_mul(orb[:, t, :], prs[t][:], dbc[:])
                    else:
                        nc.scalar.mul(orb[:, t, :], prs[t][:], dbc[:])
                i0 = gb * GG * P
                oq = nc.sync if gb % 2 == 0 else nc.scalar
                oq.dma_start(
                    out=out[i0:i0 + GG * P, 0:K].rearrange(
                        "(s p) c -> p s c", p=P),
                    in_=orb[:],
                )

    nc.compile()
    return nc


def _get_nc(d_rows):
    if d_rows not in _built:
        _built[d_rows] = _build(d_rows)
    return _built[d_rows]


def _q8(a, s):
    return np.clip(a * s, -240.0, 240.0).astype(E4)


def _run(x, W, b, trace=False, trace_cores=None):
    from concourse.bass_utils import run_bass_kernel_spmd

    x = np.ascontiguousarray(x, dtype=np.float32)
    W = np.ascontiguousarray(W, dtype=np.float32)
    b = np.asarray(b, dtype=np.float32)
    if np.any(b):
        d_rows = 1152  # pad contraction: extra ones-row in x picks up b from W
        WT_full = np.zeros((d_rows, 4 * K), np.float32)
        WT_full[:D_IN] = W.T
        WT_full[D_IN] = b
    else:
        d_rows = D_IN
        WT_full = np.ascontiguousarray(W.T)
    DT = d_rows // P
    w8u = np.ascontiguousarray(
        _q8(WT_full[:, 0:K], S_W).reshape(DT, P, K).transpose(1, 0, 2))
    w8vz = np.ascontiguousarray(
        _q8(WT_full[:, K:3 * K], S_W).reshape(DT, P, 2 * K).transpose(1, 0, 2))
    wtt = np.ascontiguousarray(
        WT_full[:, 3 * K:].astype(BF16).reshape(DT, P, K).transpose(1, 0, 2))
    nc = _get_nc(d_rows)
    in_maps = []
    for c in range(NCORES):
        xs = x[c * NLOC:(c + 1) * NLOC]
        if d_rows == D_IN:
            xTs = np.ascontiguousarray(xs.T)
        else:
            xTs = np.zeros((d_rows, NLOC), np.float32)
            xTs[:D_IN] = xs.T
            xTs[D_IN] = 1.0
        xb_bf = xTs.astype(BF16)
        x8f = _q8(xb_bf.astype(np.float32), S_X)
        # pack into block-contiguous layouts so every device load is one
        # [128 x 8KB] 2D DMA (the row-gather pattern runs at ~83GB/s)
        xb_p = np.ascontiguousarray(
            xb_bf.reshape(DT, P, NB, IB).transpose(2, 1, 0, 3))
        x8_p = np.ascontiguousarray(
            x8f.reshape(DT, P, NLOC // 1024, 1024).transpose(2, 1, 0, 3))
        in_maps.append({"x8": x8_p, "xb": xb_p, "w8u": w8u, "w8vz": w8vz, "wtt": wtt})
    res = run_bass_kernel_spmd(
        nc, in_maps, list(range(NCORES)),
        trace=trace, **({"trace_cores": trace_cores} if trace_cores else {}),
    )
    full = np.concatenate([res.results[c]["out"] for c in range(NCORES)], axis=0)
    return full, res


def kernel(x, W, b):
    full, _ = _run(x, W, b)
    return full


# revision 25
# speedup vs baseline: 1.0065x; 1.0065x over previous
"""Low-rank attention Trainium2 kernel (8 NeuronCores, SPMD), fp8 edition.

Math (reference):
    tmp = relu(x @ W.T + b); U,V,Z,T = split(tmp, 4, axis=1)
    norm = sum(U @ colsum(V)) / n + eps ;  D = 1/norm
    out = concat[(U @ (V.T @ Z)) * D, T]

Sharding: rows of x across 8 cores. Per-core partials (V.T@Z [k,k],
colsum(V), colsum(U)) are AllReduced on-device; each core then computes
its local U @ (VtZ) * D.

fp8 design (vs the 339us bf16 baseline):
- U, V, Z projections and V^T@Z run as fp8e4 DoubleRow matmuls (2 k-tiles
  per instruction, measured 216ns steady for moving-512 = true 2x bf16;
  LDWEIGHTS hides behind the previous matmul's streaming).
- x is quantized to fp8 on the HOST (x8 = e4m3(16x), 8MB/core, resident);
  on-device bf16->fp8 converts are not viable (only DVE writes fp8 fast).
- The T block stays bf16 (its error hits the output directly; fp8's ~2.5%
  elementwise would eat the whole 2e-2 budget). bf16 x streams through a
  rolling pool, one [1024, 512] block per T-pass block.
- Scales: x8 = 16x, W8 = 64W, vz fp8 = 32*[V|Z]; U drains unscaled (bf16,
  ACT relu scale 1/1024 + csu accum); VtZ psum = 1024 V^T Z | 32 csV.

Collective hiding (the bf16 baseline exposed ~41us of AllReduce):
- Phase 1a: V|Z + V^T@Z for ALL i-blocks first; V^T@Z accumulates across
  blocks in two PSUM chains (no per-block DVE adds). AllReduce A (the
  whole [k,k+1] x 2 payload) launches at ~1/3 of the kernel.
- Phase 1b: all U-passes; then AllReduce C (csu only, 1KB).
- T-passes follow (last TDEF read a csu-gated copy of the T-weights,
  pinning them after C's launch); phase-4 matmuls need only A's result
  (vtzr = V^T Z unscaled); the data-dependent D = 1/norm is applied at
  the phase-4 PSUM drains as a per-partition AP scale, so C's latency
  hides under the T-pass + phase-4 matmuls.
- DMA rings: x8 + T-out on sync, weights + staging + res-out on scalar,
  xb blocks + collectives on gpsimd.
"""
import sys

sys.path.insert(0, "/opt/trn_rl_repo")
import numpy as np
import ml_dtypes

BF16 = ml_dtypes.bfloat16
E4 = ml_dtypes.float8_e4m3

NCORES = 8
N_ROWS, D_IN, K = 65536, 1024, 256
NLOC = N_ROWS // NCORES      # 8192 rows per core
P = 128
IB = 512                     # i-block width
NB = NLOC // IB              # 16 blocks
EPS = 1e-6
TDEF = 6                     # T-pass blocks deferred behind AllReduce C
S_X, S_W, S_V = 16.0, 64.0, 32.0
S_R8 = 1.0 / 256  # VtZ fp8 scale (phase-4 DoubleRow)
X8CHUNKS = [(0, 512), (512, 512), (1024, 1024), (2048, 2048), (4096, 4096)]

_built = {}


def _build(d_rows):
    import concourse.bacc as bacc
    import concourse.mybir as mybir
    import concourse.tile as tile

    dt = mybir.dt
    f32, bf16, f8 = dt.float32, dt.bfloat16, dt.float8e4
    RELU = mybir.ActivationFunctionType.Relu
    DR = mybir.MatmulPerfMode.DoubleRow
    DT = d_rows // P
    KD2 = DT // 2            # DoubleRow kd-pairs
    KODD = DT % 2            # leftover plain-fp8 k-tile (bias-pad path)
    NSUB = IB // P
    SCL = 1.0 / (S_X * S_W)  # psum -> true pre-activation

    nc = bacc.Bacc("TRN2", target_bir_lowering=False, debug=False, num_devices=NCORES)
    NBX = NLOC // 1024       # x8 resident-load chunks
    x8d = nc.dram_tensor("x8", [NBX, P, DT, 1024], f8, kind="ExternalInput")
    xbd = nc.dram_tensor("xb", [NB, P, DT, IB], bf16, kind="ExternalInput")
    w8ud = nc.dram_tensor("w8u", [P, DT, K], f8, kind="ExternalInput")
    w8vzd = nc.dram_tensor("w8vz", [P, DT, 2 * K], f8, kind="ExternalInput")
    wttd = nc.dram_tensor("wtt", [P, DT, K], bf16, kind="ExternalInput")
    out = nc.dram_tensor("out", [NLOC, 2 * K], f32, kind="ExternalOutput")

    with tile.TileContext(nc) as tc:
        with (
            tc.tile_pool(name="wp", bufs=1) as wp,
            tc.tile_pool(name="xp", bufs=1) as xp,
            tc.tile_pool(name="xbp", bufs=8) as xbp,
            tc.tile_pool(name="up", bufs=1) as up,
            tc.tile_pool(name="vzp", bufs=6) as vzp,
            tc.tile_pool(name="ob", bufs=6) as ob,
            tc.tile_pool(name="acc", bufs=1) as accp,
            tc.tile_pool(name="ps", bufs=6, space="PSUM") as ps,
            tc.tile_pool(name="ps2", bufs=1, space="PSUM") as ps2,
            tc.tile_pool(name="dram", bufs=1, space="DRAM") as dram,
        ):
            # Weights (gpsimd/scalar rings), then resident x8 in per-kd
            # column chunks (sync ring, small first chunks so ib0 starts
            # early). bf16 x streams per-block via xbp below.
            w8vz = wp.tile([P, DT, 2 * K], f8, tag="w8vz")
            nc.scalar.dma_start(out=w8vz[:], in_=w8vzd[:])
            w8u = wp.tile([P, DT, K], f8, tag="w8u")
            nc.scalar.dma_start(out=w8u[:], in_=w8ud[:])
            xbts = {}

            def load_xb(ib):
                xbt = xbp.tile([P, DT, IB], bf16, tag="xb", name=f"xb{ib}")
                nc.scalar.dma_start(out=xbt[:], in_=xbd[ib])
                return xbt

            XC = 1024
            x8 = xp.tile([P, NBX, DT, XC], f8, tag="x8")
            nc.sync.dma_start(out=x8[:, 0, :, 0:IB], in_=x8d[0][:, :, 0:IB])
            nc.sync.dma_start(out=x8[:, 0, :, IB:XC], in_=x8d[0][:, :, IB:XC])
            for ci in range(1, NBX):
                q = nc.sync if ci % 2 == 0 else nc.scalar
                q.dma_start(out=x8[:, ci, :, :], in_=x8d[ci])
                if ci == 1:
                    wt = wp.tile([P, DT, K], bf16, tag="wt")
                    nc.scalar.dma_start(out=wt[:], in_=wttd[:])
                if 1 <= ci <= 7:
                    xbts[ci - 1] = load_xb(ci - 1)
            ones_row = wp.tile([1, P], f32, tag="ones_row")
            nc.vector.memset(ones_row[:], 1.0 / (S_V * S_R8))
            sync0 = accp.tile([1, 1], f32, tag="sync0")
            nc.vector.memset(sync0[:], 1.0)
            b0in = dram.tile([1, 1], f32)
            b0out = dram.tile([1, 1], f32)
            nc.scalar.dma_start(out=b0in[:], in_=sync0[:])
            nc.gpsimd.collective_compute(
                "AllReduce", mybir.AluOpType.add,
                replica_groups=[list(range(NCORES))],
                ins=[b0in.opt()], outs=[b0out.opt()],
            )

            ut8 = up.tile([P, 2, NLOC], f8, tag="ut8")
            csu_cols = [accp.tile([P, NB], f32, tag=f"csuc{h}", name=f"csuc{h}") for h in range(2)]

            def t_pass(ib, xbt, wsrc):
                """T = relu(x @ Wt): kd-outer so the moving operand (the
                T-weight block) stays fixed across the 4 row-subtile chains
                (alternating moving operands halve the PE issue rate)."""
                otb = ob.tile([P, NSUB, K], f32, tag="ob")
                pts = [ps.tile([P, K], f32, tag="work", name=f"pt{s}") for s in range(NSUB)]
                for kd in range(DT):
                    for s in range(NSUB):
                        nc.tensor.matmul(
                            pts[s][:], xbt[:, kd, s * P:(s + 1) * P],
                            wsrc[:, kd, :],
                            start=(kd == 0), stop=(kd == DT - 1),
                        )
                for s in range(NSUB):
                    if s % 2 == 0:
                        nc.vector.tensor_relu(otb[:, s, :], pts[s][:])
                    else:
                        nc.scalar.activation(otb[:, s, :], pts[s][:], RELU)
                i0 = ib * IB
                nc.sync.dma_start(
                    out=out[i0:i0 + IB, K:2 * K].rearrange(
                        "(s p) c -> p s c", p=P),
                    in_=otb[:],
                )
                return otb

            # ---- phase 1a: V|Z fp8 projection + V^T@Z PSUM chains ----
            # vz col 512 = 1.0 rides the V^T@Z matmul to produce 32*csV in
            # column 256 of the [k, k+1] chain.
            pzh = [ps2.tile([P, K + 1], f32, tag=f"pz{h}", name=f"pz{h}") for h in range(2)]

            def vtz(ib, vz_tiles):
                for h in range(2):
                    for sp in range(NSUB // 2):
                        nc.tensor.matmul(
                            pzh[h][:], vz_tiles[sp][:, :, h * P:(h + 1) * P],
                            vz_tiles[sp][:, :, K:2 * K + 1],
                            start=(ib == 0 and sp == 0),
                            stop=(ib == NB - 1 and sp == NSUB // 2 - 1),
                            perf_mode=DR,
                        )

            prev_vz = None
            for ib in range(NB):
                vz_tiles = []
                for sp in range(NSUB // 2):
                    vzt = vzp.tile([P, 2, 2 * K + 16], f8, tag="vz")
                    for s2 in range(2):
                        s = sp * 2 + s2
                        pvz = ps.tile([P, IB], f32, tag="work")
                        ci, off = ib // 2, (ib % 2) * IB + s * P
                        for k2 in range(KD2):
                            nc.tensor.matmul(
                                pvz[:],
                                x8[:, ci, 2 * k2:2 * k2 + 2, off:off + P],
                                w8vz[:, 2 * k2:2 * k2 + 2, :],
                                start=(k2 == 0), stop=(k2 == KD2 - 1 and not KODD),
                                perf_mode=DR,
                            )
                        if KODD:
                            nc.tensor.matmul(
                                pvz[:],
                                x8[:, ci, DT - 1, off:off + P],
                                w8vz[:, DT - 1, :],
                                start=False, stop=True,
                            )
                        nc.vector.tensor_scalar(
                            out=vzt[:, s2, 0:2 * K], in0=pvz[:],
                            scalar1=S_V * SCL, scalar2=0.0,
                            op0=mybir.AluOpType.mult, op1=mybir.AluOpType.max,
                        )
                    nc.vector.memset(vzt[:, :, 2 * K:2 * K + 1], 1.0)
                    vz_tiles.append(vzt)
                if prev_vz is not None:
                    vtz(ib - 1, prev_vz)
                prev_vz = vz_tiles
            vtz(NB - 1, prev_vz)

            # ---- AllReduce A: the full V^T@Z | csV payload ----
            bin_a = dram.tile([2 * P, K + 1], f32)
            bout_a = dram.tile([2 * P, K + 1], f32)
            vtzs = [accp.tile([P, K + 1], f32, tag=f"vtzs{h}", name=f"vtzs{h}") for h in range(2)]
            for h in range(2):
                nc.vector.tensor_copy(vtzs[h][:], pzh[h][:])
                nc.scalar.dma_start(out=bin_a[h * P:(h + 1) * P, :], in_=vtzs[h][:])
            nc.gpsimd.collective_compute(
                "AllReduce", mybir.AluOpType.add,
                replica_groups=[list(range(NCORES))],
                ins=[bin_a.opt()], outs=[bout_a.opt()],
            )

            # ---- phase 1b: all U-passes (fp8 DR), then AllReduce C (csu);
            # the first 4 T-passes weave in so the xb stream + T-out DMA load
            # spreads over a ~100us window instead of one 74us burst ----
            for ib in range(NB):
                if ib % 4 == 3:
                    otb_last = t_pass(ib // 4, xbts.pop(ib // 4), wt)
                for h in range(2):
                    pu = ps.tile([P, IB], f32, tag="work")
                    ci, off = ib // 2, (ib % 2) * IB
                    for k2 in range(KD2):
                        nc.tensor.matmul(
                            pu[:], w8u[:, 2 * k2:2 * k2 + 2, h * P:(h + 1) * P],
                            x8[:, ci, 2 * k2:2 * k2 + 2, off:off + IB],
                            start=(k2 == 0), stop=(k2 == KD2 - 1 and not KODD),
                            perf_mode=DR,
                        )
                    if KODD:
                        nc.tensor.matmul(
                            pu[:], w8u[:, DT - 1, h * P:(h + 1) * P],
                            x8[:, ci, DT - 1, off:off + IB],
                            start=False, stop=True,
                        )
                    nc.vector.tensor_scalar(
                        out=ut8[:, h, ib * IB:(ib + 1) * IB], in0=pu[:],
                        scalar1=S_V * SCL, scalar2=0.0,
                        op0=mybir.AluOpType.mult, op1=mybir.AluOpType.max,
                    )
                    scr = ob.tile([P, IB], bf16, tag="uscr")
                    nc.scalar.activation(
                        scr[:], pu[:], RELU, scale=S_V * SCL,
                        accum_out=csu_cols[h][:, ib:ib + 1],
                    )

            csu = [accp.tile([P, 1], f32, tag=f"csu{h}", name=f"csu{h}") for h in range(2)]
            for h in range(2):
                nc.vector.reduce_sum(csu[h][:], csu_cols[h][:], axis=mybir.AxisListType.X)
            bin_c = dram.tile([2, P], f32)
            bout_c = dram.tile([2, P], f32)
            for h in range(2):
                nc.scalar.dma_start(
                    out=bin_c[h, 0:P].rearrange("(p one) -> p one", one=1),
                    in_=csu[h][:],
                )
            nc.gpsimd.collective_compute(
                "AllReduce", mybir.AluOpType.add,
                replica_groups=[list(range(NCORES))],
                ins=[bin_c.opt()], outs=[bout_c.opt()],
            )

            # ---- T-passes (xb streamed per block; last TDEF read wt2) ----
            for ib in range(4, NB):
                xbt = xbts.pop(ib) if ib in xbts else load_xb(ib)
                otb_last = t_pass(ib, xbt, wt)

            # ---- phase 3 (pinned AFTER the last T relu): the Tile scheduler
            # otherwise hoists these into the T window and head-of-line
            # blocks the DVE/ACT queues on AllReduce A for ~40us. gate1/gateS
            # are exact constants (1.0, SCL) with a true data dependency on
            # the final T-block's staging tile.
            gate1 = accp.tile([P, 1], f32, tag="gate1")
            nc.vector.tensor_scalar(
                out=gate1[:], in0=otb_last[:, 0, 0:1], scalar1=0.0, scalar2=1.0,
                op0=mybir.AluOpType.mult, op1=mybir.AluOpType.add,
            )
            gateS = accp.tile([P, 1], f32, tag="gateS")
            nc.vector.tensor_scalar_mul(gateS[:], gate1[:], SCL * S_R8)
            vtzf = [accp.tile([P, K + 1], f32, tag=f"vtzf{h}", name=f"vtzf{h}") for h in range(2)]
            for h in range(2):
                nc.gpsimd.dma_start(out=vtzf[h][:], in_=bout_a[h * P:(h + 1) * P, :])
            vtzr8 = accp.tile([P, 2, K], f8, tag="vtzr8")
            for h in range(2):
                nc.vector.tensor_scalar_mul(vtzr8[:, h, :], vtzf[h][:, 0:K], gateS[:])

            # D = 1/(csU.csV/n + eps) applied at the phase-4 drains
            csut = accp.tile([P, 2], f32, tag="csut")
            nc.gpsimd.dma_start(out=csut[:], in_=bout_c.rearrange("t p -> p t"))
            csvt = accp.tile([P, 2], f32, tag="csvt")
            for h in range(2):
                nc.vector.tensor_scalar_mul(csvt[:, h:h + 1], vtzf[h][:, K:K + 1], gate1[:])
            pdot = ps.tile([1, 1], f32, tag="work")
            for h in range(2):
                nc.tensor.matmul(
                    pdot[:], csut[:, h:h + 1], csvt[:, h:h + 1],
                    start=(h == 0), stop=(h == 1),
                )
            dsb = accp.tile([1, 1], f32, tag="dsb")
            nc.vector.tensor_scalar(
                out=dsb[:], in0=pdot[:], scalar1=1.0 / (S_V * S_V * N_ROWS), scalar2=EPS,
                op0=mybir.AluOpType.mult, op1=mybir.AluOpType.add,
            )
            nc.vector.reciprocal(dsb[:], dsb[:])
            pb = ps.tile([P, 1], f32, tag="work")
            nc.tensor.matmul(pb[:], ones_row[:], dsb[:], start=True, stop=True)
            dbc = accp.tile([P, 1], f32, tag="dbc")
            nc.vector.tensor_copy(dbc[:], pb[:])

            # ---- phase 4: res = (U @ VtZ) * D, batched row-natural writes ----
            # h-major over groups of 4 PSUM tiles: the moving operand stays
            # fixed for the group and each start/stop pair is spread apart,
            # keeping the weight path warm. D lands at the drains (AP scale).
            GG = 4
            for gb in range(NLOC // P // GG):
                prs = [ps.tile([P, K], f32, tag="work", name=f"pr{t}") for t in range(GG)]
                for t in range(GG):
                    i0 = (gb * GG + t) * P
                    nc.tensor.matmul(
                        prs[t][:], ut8[:, :, i0:i0 + P], vtzr8[:],
                        start=True, stop=True, perf_mode=DR,
                    )
                orb = ob.tile([P, GG, K], f32, tag="ob")
                for t in range(GG):
                    # split PSUM->SBUF scaled copies across DVE and ACT
                    if t % 2 == 0:
                        nc.vector.tensor_scalar_mul(orb[:, t, :], prs[t][:], dbc[:])
                    else:
                        nc.scalar.mul(orb[:, t, :], prs[t][:], dbc[:])
                i0 = gb * GG * P
                oq = nc.sync if gb % 2 == 0 else nc.scalar
                oq.dma_start(
                    out=out[i0:i0 + GG * P, 0:K].rearrange(
                        "(s p) c -> p s c", p=P),
                    in_=orb[:],
                )

    nc.compile()
    return nc


def _get_nc(d_rows):
    if d_rows not in _built:
        _built[d_rows] = _build(d_rows)
    return _built[d_rows]


def _q8(a, s):
    return np.clip(a * s, -240.0, 240.0).astype(E4)


def _run(x, W, b, trace=False, trace_cores=None):
    from concourse.bass_utils import run_bass_kernel_spmd

    x = np.ascontiguousarray(x, dtype=np.float32)
    W = np.ascontiguousarray(W, dtype=np.float32)
    b = np.asarray(b, dtype=np.float32)
    if np.any(b):
        d_rows = 1152  # pad contraction: extra ones-row in x picks up b from W
        WT_full = np.zeros((d_rows, 4 * K), np.float32)
        WT_full[:D_IN] = W.T
        WT_full[D_IN] = b
    else:
        d_rows = D_IN
        WT_full = np.ascontiguousarray(W.T)
    DT = d_rows // P
    w8u = np.ascontiguousarray(
        _q8(WT_full[:, 0:K], S_W).reshape(DT, P, K).transpose(1, 0, 2))
    w8vz = np.ascontiguousarray(
        _q8(WT_full[:, K:3 * K], S_W).reshape(DT, P, 2 * K).transpose(1, 0, 2))
    wtt = np.ascontiguousarray(
        WT_full[:, 3 * K:].astype(BF16).reshape(DT, P, K).transpose(1, 0, 2))
    nc = _get_nc(d_rows)
    in_maps = []
    for c in range(NCORES):
        xs = x[c * NLOC:(c + 1) * NLOC]
        if d_rows == D_IN:
            xTs = np.ascontiguousarray(xs.T)
        else:
            xTs = np.zeros((d_rows, NLOC), np.float32)
            xTs[:D_IN] = xs.T
            xTs[D_IN] = 1.0
        xb_bf = xTs.astype(BF16)
        x8f = _q8(xb_bf.astype(np.float32), S_X)
        # pack into block-contiguous layouts so every device load is one
        # [128 x 8KB] 2D DMA (the row-gather pattern runs at ~83GB/s)
        xb_p = np.ascontiguousarray(
            xb_bf.reshape(DT, P, NB, IB).transpose(2, 1, 0, 3))
        x8_p = np.ascontiguousarray(
            x8f.reshape(DT, P, NLOC // 1024, 1024).transpose(2, 1, 0, 3))
        in_maps.append({"x8": x8_p, "xb": xb_p, "w8u": w8u, "w8vz": w8vz, "wtt": wtt})
    res = run_bass_kernel_spmd(
        nc, in_maps, list(range(NCORES)),
        trace=trace, **({"trace_cores": trace_cores} if trace_cores else {}),
    )
    full = np.concatenate([res.results[c]["out"] for c in range(NCORES)], axis=0)
    return full, res


def kernel(x, W, b):
    full, _ = _run(x, W, b)
    return full
